# revision 19
# baseline (speedup 1.0000x reference)
"""BigBird simulated attention on 8 Trainium2 NeuronCores.

Strategy
--------
B*H = 24 (batch, head) pairs are sharded 3-per-core across 8 cores (data/head
parallel). The BigBird mask is block-constant on 64x64 tiles, so the host
compresses it to a 64x64 block map and bakes a block-sparse schedule directly
into the instruction stream (the mask never goes to the device).

Per (head, q-block of 64 rows) scores are computed TRANSPOSED (S^T: k on
partitions, q on free):

  S^T[k, q] = sum_d K[k, d] Q[q, d]    (lhsT = K^T block cols, rhs = Q^T)
  P^T = exp(S^T / 8)                    (one ScalarE activation per wave)

PV runs with the V-pair as the STATIONARY operand and P^T as the moving one,
so one matmul serves every q-block of a unit at once:

  acc^T[:, q] += Vaug_pair^T @ P^T_pair[:, q]   with Vaug = [V | 1]

acc^T is [65, q]: row 64 is the softmax denominator (ones column of Vaug).
The division happens on the HOST after the un-normalized [65, q] tiles are
DMA'd back -- softmax is shift-invariant and exp can't overflow (scores
~N(0,1) after the 1/8 scale), so no max-subtraction is needed.

All matmuls are bf16 (tolerance 2e-2; this lands ~6e-3). The PE on this
toolchain serializes LDWEIGHTS with MATMUL (single weight buffer, ~90ns per
pair at these sizes), so the schedule minimizes MATMUL COUNT:
  - q-blocks are processed in device order PERM = [1..62, 0, 63],
  - the global (0,63) k-pair (every middle row attends it; appended
    host-side as resident pair #32) is ONE QK matmul + ONE PV matmul per
    7-row output bank,
  - window pairs (2t,2t+1) shared by adjacent rows 2t,2t+1 are one 128-wide
    QK + one PV,
  - full rows 0/63 (adjacent in device order, sharing all 32 aligned pairs)
    are one 128-wide QK + one PV per pair,
  - arbitrary leftover pairs are gathered host-side into per-chunk K-pair
    and V-pair tensors (kp/vp, streamed double-buffered per wave), so each
    is ONE QK matmul (128-partition) + ONE PV matmul.

Output PSUM banks hold acc^T [65, <=7 rows * 64]; banks are opened with a
[128,1] start=True dummy matmul that marks the bank's whole 2KB zero-region
pending on ALL partitions, after which every real PV matmul runs
start=False (first touch overwrites, later touches accumulate). Bank
position ranges: 8 banks of 7 middle rows, then [56..61], then [62,63]
(rows 0 and 63 share the last bank so their PV merges).

Sync: the Tile framework tracks all deps; after emission the Bacc passes
move_matmul_waits_to_ldweights + generate_event_semaphores re-establish the
TRN2 "at most one sync wait per instruction" constraint.
"""

import numpy as np

import concourse.bass as bass
import concourse.tile as tile
from concourse import mybir
from concourse.bass_utils import run_bass_kernel_spmd

B, H, S, D = 2, 12, 4096, 64
BLK = 64
NB = S // BLK            # 64 blocks per axis
DA = D + 1               # v plus ones column
NCORES = 8
HPC = B * H // NCORES    # heads per core
SCALE = 1.0 / 8.0        # 1/sqrt(64)
WAVE_CHUNKS = 24         # 24*64 cols = exactly 3 PSUM banks per score tile
NCHUNK = S // 128        # natural 128-row chunks of V
PAIR_G = NCHUNK          # resident pair index for the global (0, 63) pair
NPAIR = NCHUNK + 1

# q-block order on device: middle rows first, then the two full rows
PERM = list(range(1, NB - 1)) + [0, NB - 1]
QPOS = {r: p for p, r in enumerate(PERM)}
# output-bank position ranges: 8x7 middle rows, [56..61], then [62,63]
BANKS = [range(7 * b, 7 * b + 7) for b in range(8)] + [range(56, 62),
                                                       range(62, 64)]
NBANK = len(BANKS)
POSBANK = {p: b for b, rng in enumerate(BANKS) for p in rng}

F32 = mybir.dt.float32
BF16 = mybir.dt.bfloat16


# ----------------------------------------------------------------- schedule

def _block_mask(mask: np.ndarray) -> np.ndarray:
    m = np.asarray(mask).reshape(NB, BLK, NB, BLK)
    bm = m[:, 0, :, 0]
    assert bool(np.all(m == bm[:, None, :, None])), (
        "mask is not 64x64 block-constant; this kernel's schedule requires it"
    )
    return bm > 0


def _row_chunks(bm: np.ndarray, i: int):
    L = set(np.nonzero(bm[i])[0].tolist())
    full = len(L) == NB
    has_g = False
    if not full and 0 in L and NB - 1 in L:
        L -= {0, NB - 1}
        has_g = True
    aligned = [t for t in range(NB // 2) if 2 * t in L and 2 * t + 1 in L]
    cov = {b for t in aligned for b in (2 * t, 2 * t + 1)}
    singles = sorted(L - cov)
    spairs = [(singles[k], singles[k + 1] if k + 1 < len(singles) else None)
              for k in range(0, len(singles), 2)]
    return full, has_g, aligned, spairs


def _usize(u):
    if u[0] == "G":
        return u[2]
    return 2 if u[0] in ("W", "FR") else 1


def _ubank(u):
    if u[0] == "G":
        return POSBANK[u[1]]
    if u[0] == "W":
        return POSBANK[u[2]]
    if u[0] == "FR":
        return POSBANK[NB - 2]
    return POSBANK[u[1]]


def _build_units(bm: np.ndarray):
    info = {i: _row_chunks(bm, i) for i in range(NB)}
    wset = {}
    for t in range(NB // 2):
        r0, r1 = 2 * t, 2 * t + 1
        if (not info[r0][0] and not info[r1][0]
                and t in info[r0][2] and t in info[r1][2]):
            wset[t] = (r0, r1)
    units = []
    for b, prange in enumerate(BANKS):
        gpos = [p for p in prange if p < NB - 2]
        if gpos:
            assert all(info[PERM[p]][1] for p in gpos)
            units.append(("G", gpos[0], len(gpos)))
        for p in prange:
            r = PERM[p]
            full, has_g, aligned, spairs = info[r]
            if full:
                continue  # covered by FR units
            for t in aligned:
                if t in wset and r in wset[t]:
                    if r == wset[t][0]:
                        units.append(("W", t, QPOS[wset[t][0]]))
                else:
                    units.append(("P", p, t))
            for (gA, gB) in spairs:
                units.append(("S", p, gA, gB))
        if b == NBANK - 2:
            for t in range(NB // 2):
                units.append(("FR", t))
    return units


def _pack(units):
    """Pack units into 24-slot waves; multi-slot units must not cross an
    8-slot PSUM score-bank boundary. Lookahead picks are restricted to the
    head unit's bank or the next, so at most two output banks are ever
    accumulating at once (obpool bufs=2)."""
    pending = list(units)
    flat = []
    pos = 0
    while pending:
        rem = 8 - (pos % 8)
        head_bank = _ubank(pending[0])
        pick = None
        for idx in range(min(len(pending), 16)):
            u = pending[idx]
            if _usize(u) <= rem and _ubank(u) <= head_bank + 1:
                pick = idx
                break
        if pick is None:
            flat.append((pos, ("X",)))
            pos += 1
        else:
            u = pending.pop(pick)
            flat.append((pos, u))
            pos += _usize(u)
    waves = []
    for (p, u) in flat:
        w = p // WAVE_CHUNKS
        while len(waves) <= w:
            waves.append([])
        waves[w].append((p % WAVE_CHUNKS, u))
    return waves


def _unit_pv(u, slot):
    """PV matmuls for a unit: (pos0, pT slot0, width, source).

    source: ("v2", pair) resident, or ("vp", sidx) gathered."""
    k = u[0]
    if k == "G":
        return [(u[1], slot, u[2], ("v2", PAIR_G))]
    if k == "FR":
        return [(NB - 2, slot, 2, ("v2", u[1]))]
    if k == "W":
        p0 = u[2]
        if POSBANK[p0] == POSBANK[p0 + 1]:
            return [(p0, slot, 2, ("v2", u[1]))]
        return [(p0, slot, 1, ("v2", u[1])),
                (p0 + 1, slot + 1, 1, ("v2", u[1]))]
    if k == "P":
        return [(u[1], slot, 1, ("v2", u[2]))]
    return [(u[1], slot, 1, ("vp", u[4]))]


def _build_schedule(bm: np.ndarray):
    units = _build_units(bm)
    waves = _pack(units)
    ns = 0
    waves2 = []
    pos_chunks = np.zeros(NB, dtype=np.int64)
    for wave in waves:
        w2 = []
        for slot, u in wave:
            if u[0] == "S":
                u = u + (ns,)
                ns += 1
            w2.append((slot, u))
            if u[0] != "X":
                for (p0, s0, width, src) in _unit_pv(u, slot):
                    for j in range(width):
                        pos_chunks[p0 + j] += 1
        waves2.append(w2)
    return waves2, ns, pos_chunks


# ------------------------------------------------------------------ program

def _emit_head(tc, pools, h, waves, pos_chunks, qT_d, kT_d, v2_d, vp_d, kp_d,
               o_d):
    nc = tc.nc
    (wq, wk, wv, vppool, kppool, ppool, stpool, obpool, fpool) = pools

    qT = wq.tile([64, S], BF16, tag="qT", name=f"qT{h}")
    kT = wk.tile([64, S + 2 * BLK], BF16, tag="kT", name=f"kT{h}")
    v2 = wv.tile([128, NPAIR * DA], BF16, tag="v2", name=f"v2_{h}")
    vps = [vppool.tile([128, WAVE_CHUNKS * DA], BF16, tag=f"vp{j}",
                       name=f"vp{j}h{h}") for j in range(2)]
    kps = [kppool.tile([64, WAVE_CHUNKS * 128], BF16, tag=f"kp{j}",
                       name=f"kp{j}h{h}") for j in range(2)]
    nc.gpsimd.dma_start(out=qT, in_=qT_d[h])
    nc.gpsimd.dma_start(out=kT, in_=kT_d[h])
    nc.gpsimd.dma_start(out=v2, in_=v2_d[h])

    ob_tiles = {}
    slots_left = {b: int(sum(pos_chunks[p] for p in rng))
                  for b, rng in enumerate(BANKS)}

    def _finalize(bank):
        ob = ob_tiles.pop(bank)
        nq = len(BANKS[bank])
        oc = fpool.tile([128, 7 * BLK], F32, tag="oc", name=f"och{h}_{bank}")
        nc.vector.tensor_copy(oc[0:DA, : nq * BLK], ob[0:DA, : nq * BLK])
        nc.gpsimd.dma_start(out=o_d[h][bank][:, :],
                            in_=oc[0:DA, : nq * BLK])

    def emit_qk(wave, st, kp, k0):
        for slot, u in wave:
            c0 = slot * BLK
            kind = u[0]
            if kind == "X":
                continue
            if kind == "G":
                p0, g = u[1], u[2]
                nc.tensor.matmul(
                    st[:, c0: c0 + g * BLK],
                    lhsT=kT[0:64, 2 * PAIR_G * BLK: (2 * PAIR_G + 2) * BLK],
                    rhs=qT[0:64, p0 * BLK: (p0 + g) * BLK],
                    start=True, stop=True, skip_group_check=True)
            elif kind in ("W", "FR"):
                t = u[1]
                p0 = u[2] if kind == "W" else NB - 2
                nc.tensor.matmul(
                    st[:, c0: c0 + 2 * BLK],
                    lhsT=kT[0:64, 2 * t * BLK: (2 * t + 2) * BLK],
                    rhs=qT[0:64, p0 * BLK: (p0 + 2) * BLK],
                    start=True, stop=True, skip_group_check=True)
            elif kind == "P":
                p, t = u[1], u[2]
                nc.tensor.matmul(
                    st[:, c0: c0 + BLK],
                    lhsT=kT[0:64, 2 * t * BLK: (2 * t + 2) * BLK],
                    rhs=qT[0:64, p * BLK: (p + 1) * BLK],
                    start=True, stop=True, skip_group_check=True)
            else:  # S: gathered K pair, one 128-partition matmul
                p, li = u[1], u[4] - k0
                nc.tensor.matmul(
                    st[:, c0: c0 + BLK],
                    lhsT=kp[0:64, li * 128: (li + 1) * 128],
                    rhs=qT[0:64, p * BLK: (p + 1) * BLK],
                    start=True, stop=True, skip_group_check=True)

    def emit_pv(wave, pT, vp, s0):
        for slot, u in wave:
            if u[0] == "X":
                continue
            for (p0, ps, width, src) in _unit_pv(u, slot):
                bank = POSBANK[p0]
                if bank not in ob_tiles:
                    ob = obpool.tile([128, 512], F32, tag="ob",
                                     name=f"obh{h}_{bank}")
                    ob_tiles[bank] = ob
                    # start=True [128,1]: marks the whole 2KB zero-region
                    # pending on ALL partitions; real PVs run start=False
                    nc.tensor.matmul(
                        ob[0:128, 508:509], lhsT=qT[0:1, 0:128],
                        rhs=qT[0:1, 0:1],
                        start=True, stop=True, skip_group_check=True)
                ob = ob_tiles[bank]
                if src[0] == "v2":
                    lhsT = v2[:, src[1] * DA: (src[1] + 1) * DA]
                else:
                    li = src[1] - s0
                    lhsT = vp[:, li * DA: (li + 1) * DA]
                col0 = (p0 - BANKS[bank].start) * BLK
                slots_left[bank] -= width
                nc.tensor.matmul(
                    ob[0:DA, col0: col0 + width * BLK],
                    lhsT=lhsT,
                    rhs=pT[:, ps * BLK: (ps + width) * BLK],
                    start=False, stop=(slots_left[bank] == 0),
                    skip_group_check=True)
                if slots_left[bank] == 0:
                    # eager: free the psum bank before the next one opens
                    del slots_left[bank]
                    _finalize(bank)

    def wave_s0(j):
        sidxs = [u[4] for _, u in waves[j] if u[0] == "S"]
        return min(sidxs) if sidxs else 0

    def load_sp(j):
        sidxs = [u[4] for _, u in waves[j] if u[0] == "S"]
        if not sidxs:
            return
        s0, scnt = min(sidxs), len(sidxs)
        assert sidxs == list(range(s0, s0 + scnt))
        nc.gpsimd.dma_start(
            out=vps[j % 2][:, : scnt * DA].rearrange(
                "p (s c) -> p s c", c=DA),
            in_=vp_d[h, s0: s0 + scnt].rearrange("s p c -> p s c"))
        nc.gpsimd.dma_start(
            out=kps[j % 2][:, : scnt * 128].rearrange(
                "p (s c) -> p s c", c=128),
            in_=kp_d[h, s0: s0 + scnt].rearrange("s p c -> p s c"))

    load_sp(0)
    if len(waves) > 1:
        load_sp(1)
    prev = None
    for w, wave in enumerate(waves):
        st = stpool.tile([128, WAVE_CHUNKS * BLK], F32, tag="st",
                         name=f"sth{h}_{w}")
        emit_qk(wave, st, kps[w % 2], wave_s0(w))
        pT = ppool.tile([128, WAVE_CHUNKS * BLK], BF16, tag="pT",
                        name=f"pTh{h}_{w}")
        ncols = (wave[-1][0] + _usize(wave[-1][1])) * BLK
        nc.scalar.activation(
            out=pT[:, :ncols], in_=st[:, :ncols],
            func=mybir.ActivationFunctionType.Exp, scale=SCALE)
        if prev is not None:
            emit_pv(waves[prev[0]], prev[1], prev[2], wave_s0(prev[0]))
            if w + 1 < len(waves):
                load_sp(w + 1)
        prev = (w, pT, vps[w % 2])
    emit_pv(waves[prev[0]], prev[1], prev[2], wave_s0(prev[0]))
    for bank in sorted(ob_tiles):
        slots_left.pop(bank, None)
        _finalize(bank)


def _build_program(bm: np.ndarray):
    import os as _os
    hpc = int(_os.environ.get("BB_HPC", HPC))
    waves, ns, pos_chunks = _build_schedule(bm)
    nc = bass.Bass("TRN2", target_bir_lowering=False, debug=False,
                   enable_asserts=False)
    qT_d = nc.dram_tensor("qT", [HPC, 64, S], BF16, kind="ExternalInput")
    kT_d = nc.dram_tensor("kT", [HPC, 64, S + 2 * BLK], BF16,
                          kind="ExternalInput")
    v2_d = nc.dram_tensor("v2", [HPC, 128, NPAIR * DA], BF16,
                          kind="ExternalInput")
    vp_d = nc.dram_tensor("vp", [HPC, max(ns, 1), 128, DA], BF16,
                          kind="ExternalInput")
    kp_d = nc.dram_tensor("kp", [HPC, max(ns, 1), 64, 128], BF16,
                          kind="ExternalInput")
    o_d = [[nc.dram_tensor(f"o_{hh}_{bb}", [DA, len(BANKS[bb]) * BLK], F32,
                           kind="ExternalOutput")
            for bb in range(NBANK)] for hh in range(HPC)]

    with tile.TileContext(nc) as tc:
        with (
            tc.tile_pool(name="wq", bufs=HPC) as wq,
            tc.tile_pool(name="wk", bufs=HPC) as wk,
            tc.tile_pool(name="wv", bufs=HPC) as wv,
            tc.tile_pool(name="vpp", bufs=HPC) as vppool,
            tc.tile_pool(name="kpp", bufs=HPC) as kppool,
            tc.tile_pool(name="pT", bufs=3) as ppool,
            tc.tile_pool(name="st", bufs=2, space="PSUM") as stpool,
            tc.tile_pool(name="ob", bufs=2, space="PSUM") as obpool,
            tc.tile_pool(name="fin", bufs=3) as fpool,
        ):
            pools = (wq, wk, wv, vppool, kppool, ppool, stpool, obpool,
                     fpool)
            for hh in range(hpc):
                _emit_head(tc, pools, hh, waves, pos_chunks,
                           qT_d, kT_d, v2_d, vp_d, kp_d, o_d)
    import bass_rust as _bass_rust
    _bass_rust.move_matmul_waits_to_ldweights(nc.m)
    _bass_rust.generate_event_semaphores(nc)
    return nc


_CACHE = {}


def _get_program(bm: np.ndarray):
    key = bm.tobytes()
    if key not in _CACHE:
        _CACHE[key] = _build_program(bm)
    return _CACHE[key]


# -------------------------------------------------------------------- entry

def _prep_inputs(q, k, v, waves, ns):
    import ml_dtypes
    bf16 = ml_dtypes.bfloat16
    q = np.ascontiguousarray(np.asarray(q), dtype=np.float32)
    k = np.ascontiguousarray(np.asarray(k), dtype=np.float32)
    v = np.ascontiguousarray(np.asarray(v), dtype=np.float32)
    qT = q.reshape(B * H, S, D).transpose(0, 2, 1).astype(bf16)
    qTr = np.ascontiguousarray(
        qT.reshape(B * H, D, NB, BLK)[:, :, PERM, :].reshape(B * H, D, S))
    kT = k.reshape(B * H, S, D).transpose(0, 2, 1).astype(bf16)
    kTe = np.ascontiguousarray(np.concatenate(
        [kT, kT[:, :, :BLK], kT[:, :, (NB - 1) * BLK:]], axis=2))
    vA = np.concatenate(
        [v.reshape(B * H, S, D),
         np.ones((B * H, S, 1), dtype=np.float32)], axis=2).astype(bf16)
    v2 = vA.reshape(B * H, NCHUNK, 128, DA)
    vblk = vA.reshape(B * H, NB, BLK, DA)
    pair_g = np.concatenate([vblk[:, 0], vblk[:, NB - 1]], axis=1)
    v2e = np.ascontiguousarray(
        np.concatenate([v2, pair_g[:, None]], axis=1)
        .transpose(0, 2, 1, 3).reshape(B * H, 128, NPAIR * DA))
    kblk = kTe[:, :, :S].reshape(B * H, D, NB, BLK)
    vp = np.zeros((B * H, max(ns, 1), 128, DA), dtype=bf16)
    kp = np.zeros((B * H, max(ns, 1), 64, 128), dtype=bf16)
    for wave in waves:
        for slot, u in wave:
            if u[0] != "S":
                continue
            for half, g in enumerate(u[2:4]):
                if g is not None:
                    vp[:, u[4], half * 64: half * 64 + 64, :] = vblk[:, g]
                    kp[:, u[4], :, half * 64: half * 64 + 64] = \
                        kblk[:, :, g, :]
    return qTr, kTe, v2e, vp, kp


def _run(inputs, trace=False):
    q, k, v, mask = inputs["q"], inputs["k"], inputs["v"], inputs["mask"]
    bm = _block_mask(mask)
    nc = _get_program(bm)
    waves, ns, _ = _build_schedule(bm)
    qTr, kTe, v2e, vp, kp = _prep_inputs(q, k, v, waves, ns)
    in_maps = []
    for c in range(NCORES):
        sl = slice(c * HPC, (c + 1) * HPC)
        in_maps.append({
            "qT": np.ascontiguousarray(qTr[sl]),
            "kT": np.ascontiguousarray(kTe[sl]),
            "v2": np.ascontiguousarray(v2e[sl]),
            "vp": np.ascontiguousarray(vp[sl]),
            "kp": np.ascontiguousarray(kp[sl]),
        })
    bkr = run_bass_kernel_spmd(nc, in_maps, list(range(NCORES)), trace=trace)
    pieces = []
    for r in bkr.results:
        for hh in range(HPC):
            accT = np.concatenate(
                [np.asarray(r[f"o_{hh}_{bb}"]) for bb in range(NBANK)],
                axis=1)                       # [65, 64*64] by position
            accT = accT.reshape(DA, NB, BLK)
            out_pos = (accT[:D] / accT[D:]).transpose(1, 2, 0)  # [NB,BLK,D]
            inv = np.asarray([QPOS[r2] for r2 in range(NB)])
            pieces.append(out_pos[inv].reshape(S, D))
    out = np.stack(pieces, axis=0).reshape(B, H, S, D).astype(np.float32)
    return out, bkr


def kernel(**inputs):
    out, _ = _run(inputs, trace=False)
    return out


# revision 24
# speedup vs baseline: 1.2317x; 1.2317x over previous
"""BigBird simulated attention on 8 Trainium2 NeuronCores.

Strategy
--------
B*H = 24 (batch, head) pairs are sharded 3-per-core across 8 cores (data/head
parallel). The BigBird mask is block-constant on 64x64 tiles, so the host
compresses it to a 64x64 block map and bakes a block-sparse schedule directly
into the instruction stream (the mask never goes to the device).

Per (head, q-block of 64 rows) scores are computed TRANSPOSED (S^T: k on
partitions, q on free):

  S^T[k, q] = sum_d K[k, d] Q[q, d]    (lhsT = K^T block cols, rhs = Q^T)
  P^T = exp(S^T / 8)                    (one ScalarE activation per wave)

PV runs with the V-pair as the STATIONARY operand and P^T as the moving one,
so one matmul serves every q-block of a unit at once:

  acc^T[:, q] += Vaug_pair^T @ P^T_pair[:, q]   with Vaug = [V | 1]

acc^T is [65, q]: row 64 is the softmax denominator (ones column of Vaug).
The division happens on the HOST after the un-normalized [65, q] tiles are
DMA'd back -- softmax is shift-invariant and exp can't overflow (scores
~N(0,1) after the 1/8 scale), so no max-subtraction is needed.

All matmuls are bf16 (tolerance 2e-2; this lands ~6e-3). The PE on this
toolchain serializes LDWEIGHTS with MATMUL (single weight buffer, ~90ns per
pair at these sizes), so the schedule minimizes MATMUL COUNT:
  - q-blocks are processed in device order PERM = [1..62, 0, 63],
  - the global (0,63) k-pair (every middle row attends it; appended
    host-side as resident pair #32) is ONE QK matmul + ONE PV matmul per
    7-row output bank,
  - window pairs (2t,2t+1) shared by adjacent rows 2t,2t+1 are one 128-wide
    QK + one PV,
  - full rows 0/63 (adjacent in device order, sharing all 32 aligned pairs)
    are one 128-wide QK + one PV per pair,
  - arbitrary leftover pairs are gathered host-side into per-chunk K-pair
    and V-pair tensors (kp/vp, streamed double-buffered per wave), so each
    is ONE QK matmul (128-partition) + ONE PV matmul.

Output PSUM banks hold acc^T [65, <=7 rows * 64]; banks are opened with a
[128,1] start=True dummy matmul that marks the bank's whole 2KB zero-region
pending on ALL partitions, after which every real PV matmul runs
start=False (first touch overwrites, later touches accumulate). Bank
position ranges: 8 banks of 7 middle rows, then [56..61], then [62,63]
(rows 0 and 63 share the last bank so their PV merges).

Sync: the Tile framework tracks all deps; after emission the Bacc passes
move_matmul_waits_to_ldweights + generate_event_semaphores re-establish the
TRN2 "at most one sync wait per instruction" constraint.
"""

import numpy as np

import concourse.bass as bass
import concourse.tile as tile
from concourse import mybir
from concourse.bass_utils import run_bass_kernel_spmd

B, H, S, D = 2, 12, 4096, 64
BLK = 64
NB = S // BLK            # 64 blocks per axis
DA = D + 1               # v plus ones column
NCORES = 8
HPC = B * H // NCORES    # heads per core
SCALE = 1.0 / 8.0        # 1/sqrt(64)
WAVE_CHUNKS = 24         # 24*64 cols = exactly 3 PSUM banks per score tile
NCHUNK = S // 128        # natural 128-row chunks of V
PAIR_G = NCHUNK          # resident pair index for the global (0, 63) pair
NPAIR = NCHUNK + 1

# q-block order on device: middle rows first, then the two full rows
PERM = list(range(1, NB - 1)) + [0, NB - 1]
QPOS = {r: p for p, r in enumerate(PERM)}
# output-bank position ranges: 8x7 middle rows, [56..61], then [62,63]
BANKS = [range(7 * b, 7 * b + 7) for b in range(8)] + [range(56, 62),
                                                       range(62, 64)]
NBANK = len(BANKS)
POSBANK = {p: b for b, rng in enumerate(BANKS) for p in rng}

F32 = mybir.dt.float32
BF16 = mybir.dt.bfloat16


# ----------------------------------------------------------------- schedule

def _block_mask(mask: np.ndarray) -> np.ndarray:
    m = np.asarray(mask).reshape(NB, BLK, NB, BLK)
    bm = m[:, 0, :, 0]
    assert bool(np.all(m == bm[:, None, :, None])), (
        "mask is not 64x64 block-constant; this kernel's schedule requires it"
    )
    return bm > 0


def _row_chunks(bm: np.ndarray, i: int):
    L = set(np.nonzero(bm[i])[0].tolist())
    full = len(L) == NB
    has_g = False
    if not full and 0 in L and NB - 1 in L:
        L -= {0, NB - 1}
        has_g = True
    aligned = [t for t in range(NB // 2) if 2 * t in L and 2 * t + 1 in L]
    cov = {b for t in aligned for b in (2 * t, 2 * t + 1)}
    singles = sorted(L - cov)
    spairs = [(singles[k], singles[k + 1] if k + 1 < len(singles) else None)
              for k in range(0, len(singles), 2)]
    return full, has_g, aligned, spairs


def _usize(u):
    if u[0] == "G":
        return u[2]
    return 2 if u[0] in ("W", "FR") else 1


def _ubank(u):
    if u[0] == "G":
        return POSBANK[u[1]]
    if u[0] == "W":
        return POSBANK[u[2]]
    if u[0] == "FR":
        return POSBANK[NB - 2]
    return POSBANK[u[1]]


def _build_units(bm: np.ndarray):
    info = {i: _row_chunks(bm, i) for i in range(NB)}
    wset = {}
    for t in range(NB // 2):
        r0, r1 = 2 * t, 2 * t + 1
        if (not info[r0][0] and not info[r1][0]
                and t in info[r0][2] and t in info[r1][2]):
            wset[t] = (r0, r1)
    units = []
    for b, prange in enumerate(BANKS):
        gpos = [p for p in prange if p < NB - 2]
        if gpos:
            assert all(info[PERM[p]][1] for p in gpos)
            units.append(("G", gpos[0], len(gpos)))
        for p in prange:
            r = PERM[p]
            full, has_g, aligned, spairs = info[r]
            if full:
                continue  # covered by FR units
            for t in aligned:
                if t in wset and r in wset[t]:
                    if r == wset[t][0]:
                        units.append(("W", t, QPOS[wset[t][0]]))
                else:
                    units.append(("P", p, t))
            for (gA, gB) in spairs:
                units.append(("S", p, gA, gB))
        if b == NBANK - 2:
            for t in range(NB // 2):
                units.append(("FR", t))
    return units


def _pack(units):
    """Pack units into 24-slot waves; multi-slot units must not cross an
    8-slot PSUM score-bank boundary. Lookahead picks are restricted to the
    head unit's bank or the next, so at most two output banks are ever
    accumulating at once (obpool bufs=2)."""
    pending = list(units)
    flat = []
    pos = 0
    while pending:
        rem = 8 - (pos % 8)
        head_bank = _ubank(pending[0])
        pick = None
        for idx in range(min(len(pending), 16)):
            u = pending[idx]
            if _usize(u) <= rem and _ubank(u) <= head_bank + 1:
                pick = idx
                break
        if pick is None:
            flat.append((pos, ("X",)))
            pos += 1
        else:
            u = pending.pop(pick)
            flat.append((pos, u))
            pos += _usize(u)
    waves = []
    for (p, u) in flat:
        w = p // WAVE_CHUNKS
        while len(waves) <= w:
            waves.append([])
        waves[w].append((p % WAVE_CHUNKS, u))
    return waves


def _unit_pv(u, slot):
    """PV matmuls for a unit: (pos0, pT slot0, width, source).

    source: ("v2", pair) resident, or ("vp", sidx) gathered."""
    k = u[0]
    if k == "G":
        return [(u[1], slot, u[2], ("v2", PAIR_G))]
    if k == "FR":
        return [(NB - 2, slot, 2, ("v2", u[1]))]
    if k == "W":
        p0 = u[2]
        if POSBANK[p0] == POSBANK[p0 + 1]:
            return [(p0, slot, 2, ("v2", u[1]))]
        return [(p0, slot, 1, ("v2", u[1])),
                (p0 + 1, slot + 1, 1, ("v2", u[1]))]
    if k == "P":
        return [(u[1], slot, 1, ("v2", u[2]))]
    return [(u[1], slot, 1, ("vp", u[4]))]


def _build_schedule(bm: np.ndarray):
    units = _build_units(bm)
    waves = _pack(units)
    ns = 0
    waves2 = []
    pos_chunks = np.zeros(NB, dtype=np.int64)
    for wave in waves:
        w2 = []
        for slot, u in wave:
            if u[0] == "S":
                u = u + (ns,)
                ns += 1
            w2.append((slot, u))
            if u[0] != "X":
                for (p0, s0, width, src) in _unit_pv(u, slot):
                    for j in range(width):
                        pos_chunks[p0 + j] += 1
        waves2.append(w2)
    return waves2, ns, pos_chunks


# ------------------------------------------------------------------ program

def _emit_head(tc, pools, h, waves, pos_chunks, qT_d, kT_d, v2_d, vp_d, kp_d,
               o_d):
    nc = tc.nc
    (wq, wk, wv, vppool, kppool, ppool, stpool, obpool, fpool) = pools

    qT = wq.tile([64, S], BF16, tag="qT", name=f"qT{h}")
    kT = wk.tile([64, S + 2 * BLK], BF16, tag="kT", name=f"kT{h}")
    v2 = wv.tile([128, NPAIR * DA], BF16, tag="v2", name=f"v2_{h}")
    vps = [vppool.tile([128, WAVE_CHUNKS * DA], BF16, tag=f"vp{j}",
                       name=f"vp{j}h{h}") for j in range(2)]
    kps = [kppool.tile([64, WAVE_CHUNKS * 128], BF16, tag=f"kp{j}",
                       name=f"kp{j}h{h}") for j in range(2)]
    nc.sync.dma_start(out=qT, in_=qT_d[h])
    nc.sync.dma_start(out=kT, in_=kT_d[h])
    nc.sync.dma_start(out=v2, in_=v2_d[h])

    ob_tiles = {}
    slots_left = {b: int(sum(pos_chunks[p] for p in rng))
                  for b, rng in enumerate(BANKS)}

    def _finalize(bank):
        ob = ob_tiles.pop(bank)
        nq = len(BANKS[bank])
        oc = fpool.tile([128, 7 * BLK], F32, tag="oc", name=f"och{h}_{bank}")
        nc.vector.tensor_copy(oc[0:DA, : nq * BLK], ob[0:DA, : nq * BLK])
        nc.scalar.dma_start(out=o_d[h][bank][:, :],
                            in_=oc[0:DA, : nq * BLK])

    def emit_qk(wave, st, kp, k0):
        for slot, u in wave:
            c0 = slot * BLK
            kind = u[0]
            if kind == "X":
                continue
            if kind == "G":
                p0, g = u[1], u[2]
                nc.tensor.matmul(
                    st[:, c0: c0 + g * BLK],
                    lhsT=kT[0:64, 2 * PAIR_G * BLK: (2 * PAIR_G + 2) * BLK],
                    rhs=qT[0:64, p0 * BLK: (p0 + g) * BLK],
                    start=True, stop=True, skip_group_check=True)
            elif kind in ("W", "FR"):
                t = u[1]
                p0 = u[2] if kind == "W" else NB - 2
                nc.tensor.matmul(
                    st[:, c0: c0 + 2 * BLK],
                    lhsT=kT[0:64, 2 * t * BLK: (2 * t + 2) * BLK],
                    rhs=qT[0:64, p0 * BLK: (p0 + 2) * BLK],
                    start=True, stop=True, skip_group_check=True)
            elif kind == "P":
                p, t = u[1], u[2]
                nc.tensor.matmul(
                    st[:, c0: c0 + BLK],
                    lhsT=kT[0:64, 2 * t * BLK: (2 * t + 2) * BLK],
                    rhs=qT[0:64, p * BLK: (p + 1) * BLK],
                    start=True, stop=True, skip_group_check=True)
            else:  # S: gathered K pair, one 128-partition matmul
                p, li = u[1], u[4] - k0
                nc.tensor.matmul(
                    st[:, c0: c0 + BLK],
                    lhsT=kp[0:64, li * 128: (li + 1) * 128],
                    rhs=qT[0:64, p * BLK: (p + 1) * BLK],
                    start=True, stop=True, skip_group_check=True)

    def emit_pv(wave, pT, vp, s0):
        for slot, u in wave:
            if u[0] == "X":
                continue
            for (p0, ps, width, src) in _unit_pv(u, slot):
                bank = POSBANK[p0]
                if bank not in ob_tiles:
                    ob = obpool.tile([128, 512], F32, tag="ob",
                                     name=f"obh{h}_{bank}")
                    ob_tiles[bank] = ob
                    # start=True [128,1]: marks the whole 2KB zero-region
                    # pending on ALL partitions; real PVs run start=False
                    nc.tensor.matmul(
                        ob[0:128, 508:509], lhsT=qT[0:1, 0:128],
                        rhs=qT[0:1, 0:1],
                        start=True, stop=True, skip_group_check=True)
                ob = ob_tiles[bank]
                if src[0] == "v2":
                    lhsT = v2[:, src[1] * DA: (src[1] + 1) * DA]
                else:
                    li = src[1] - s0
                    lhsT = vp[:, li * DA: (li + 1) * DA]
                col0 = (p0 - BANKS[bank].start) * BLK
                slots_left[bank] -= width
                nc.tensor.matmul(
                    ob[0:DA, col0: col0 + width * BLK],
                    lhsT=lhsT,
                    rhs=pT[:, ps * BLK: (ps + width) * BLK],
                    start=False, stop=(slots_left[bank] == 0),
                    skip_group_check=True)
                if slots_left[bank] == 0:
                    # eager: free the psum bank before the next one opens
                    del slots_left[bank]
                    _finalize(bank)

    def wave_s0(j):
        sidxs = [u[4] for _, u in waves[j] if u[0] == "S"]
        return min(sidxs) if sidxs else 0

    def load_sp(j):
        sidxs = [u[4] for _, u in waves[j] if u[0] == "S"]
        if not sidxs:
            return
        s0, scnt = min(sidxs), len(sidxs)
        assert sidxs == list(range(s0, s0 + scnt))
        # contiguous column slices: one ~1-2KB run per partition
        nc.gpsimd.dma_start(
            out=vps[j % 2][:, : scnt * DA],
            in_=vp_d[h][:, s0 * DA: (s0 + scnt) * DA])
        nc.gpsimd.dma_start(
            out=kps[j % 2][:, : scnt * 128],
            in_=kp_d[h][:, s0 * 128: (s0 + scnt) * 128])

    load_sp(0)
    if len(waves) > 1:
        load_sp(1)
    prev = None
    for w, wave in enumerate(waves):
        st = stpool.tile([128, WAVE_CHUNKS * BLK], F32, tag="st",
                         name=f"sth{h}_{w}")
        emit_qk(wave, st, kps[w % 2], wave_s0(w))
        pT = ppool.tile([128, WAVE_CHUNKS * BLK], BF16, tag="pT",
                        name=f"pTh{h}_{w}")
        ncols = (wave[-1][0] + _usize(wave[-1][1])) * BLK
        nc.scalar.activation(
            out=pT[:, :ncols], in_=st[:, :ncols],
            func=mybir.ActivationFunctionType.Exp, scale=SCALE)
        if prev is not None:
            emit_pv(waves[prev[0]], prev[1], prev[2], wave_s0(prev[0]))
            if w + 1 < len(waves):
                load_sp(w + 1)
        prev = (w, pT, vps[w % 2])
    emit_pv(waves[prev[0]], prev[1], prev[2], wave_s0(prev[0]))
    for bank in sorted(ob_tiles):
        slots_left.pop(bank, None)
        _finalize(bank)


def _build_program(bm: np.ndarray):
    import os as _os
    hpc = int(_os.environ.get("BB_HPC", HPC))
    waves, ns, pos_chunks = _build_schedule(bm)
    nc = bass.Bass("TRN2", target_bir_lowering=False, debug=False,
                   enable_asserts=False)
    qT_d = nc.dram_tensor("qT", [HPC, 64, S], BF16, kind="ExternalInput")
    kT_d = nc.dram_tensor("kT", [HPC, 64, S + 2 * BLK], BF16,
                          kind="ExternalInput")
    v2_d = nc.dram_tensor("v2", [HPC, 128, NPAIR * DA], BF16,
                          kind="ExternalInput")
    vp_d = nc.dram_tensor("vp", [HPC, 128, max(ns, 1) * DA], BF16,
                          kind="ExternalInput")
    kp_d = nc.dram_tensor("kp", [HPC, 64, max(ns, 1) * 128], BF16,
                          kind="ExternalInput")
    o_d = [[nc.dram_tensor(f"o_{hh}_{bb}", [DA, len(BANKS[bb]) * BLK], F32,
                           kind="ExternalOutput")
            for bb in range(NBANK)] for hh in range(HPC)]

    with tile.TileContext(nc) as tc:
        with (
            tc.tile_pool(name="wq", bufs=HPC) as wq,
            tc.tile_pool(name="wk", bufs=HPC) as wk,
            tc.tile_pool(name="wv", bufs=HPC) as wv,
            tc.tile_pool(name="vpp", bufs=HPC) as vppool,
            tc.tile_pool(name="kpp", bufs=HPC) as kppool,
            tc.tile_pool(name="pT", bufs=3) as ppool,
            tc.tile_pool(name="st", bufs=2, space="PSUM") as stpool,
            tc.tile_pool(name="ob", bufs=2, space="PSUM") as obpool,
            tc.tile_pool(name="fin", bufs=3) as fpool,
        ):
            pools = (wq, wk, wv, vppool, kppool, ppool, stpool, obpool,
                     fpool)
            for hh in range(hpc):
                _emit_head(tc, pools, hh, waves, pos_chunks,
                           qT_d, kT_d, v2_d, vp_d, kp_d, o_d)
    import bass_rust as _bass_rust
    _bass_rust.move_matmul_waits_to_ldweights(nc.m)
    _bass_rust.generate_event_semaphores(nc)
    return nc


_CACHE = {}


def _get_program(bm: np.ndarray):
    key = bm.tobytes()
    if key not in _CACHE:
        _CACHE[key] = _build_program(bm)
    return _CACHE[key]


# -------------------------------------------------------------------- entry

def _prep_inputs(q, k, v, waves, ns):
    import ml_dtypes
    bf16 = ml_dtypes.bfloat16
    q = np.ascontiguousarray(np.asarray(q), dtype=np.float32)
    k = np.ascontiguousarray(np.asarray(k), dtype=np.float32)
    v = np.ascontiguousarray(np.asarray(v), dtype=np.float32)
    qT = q.reshape(B * H, S, D).transpose(0, 2, 1).astype(bf16)
    qTr = np.ascontiguousarray(
        qT.reshape(B * H, D, NB, BLK)[:, :, PERM, :].reshape(B * H, D, S))
    kT = k.reshape(B * H, S, D).transpose(0, 2, 1).astype(bf16)
    kTe = np.ascontiguousarray(np.concatenate(
        [kT, kT[:, :, :BLK], kT[:, :, (NB - 1) * BLK:]], axis=2))
    vA = np.concatenate(
        [v.reshape(B * H, S, D),
         np.ones((B * H, S, 1), dtype=np.float32)], axis=2).astype(bf16)
    v2 = vA.reshape(B * H, NCHUNK, 128, DA)
    vblk = vA.reshape(B * H, NB, BLK, DA)
    pair_g = np.concatenate([vblk[:, 0], vblk[:, NB - 1]], axis=1)
    v2e = np.ascontiguousarray(
        np.concatenate([v2, pair_g[:, None]], axis=1)
        .transpose(0, 2, 1, 3).reshape(B * H, 128, NPAIR * DA))
    kblk = kTe[:, :, :S].reshape(B * H, D, NB, BLK)
    vp = np.zeros((B * H, max(ns, 1), 128, DA), dtype=bf16)
    kp = np.zeros((B * H, max(ns, 1), 64, 128), dtype=bf16)
    for wave in waves:
        for slot, u in wave:
            if u[0] != "S":
                continue
            for half, g in enumerate(u[2:4]):
                if g is not None:
                    vp[:, u[4], half * 64: half * 64 + 64, :] = vblk[:, g]
                    kp[:, u[4], :, half * 64: half * 64 + 64] = \
                        kblk[:, :, g, :]
    # partition-major layouts so per-wave loads are contiguous column runs
    vp = np.ascontiguousarray(
        vp.transpose(0, 2, 1, 3).reshape(B * H, 128, -1))
    kp = np.ascontiguousarray(
        kp.transpose(0, 2, 1, 3).reshape(B * H, 64, -1))
    return qTr, kTe, v2e, vp, kp


def _run(inputs, trace=False):
    q, k, v, mask = inputs["q"], inputs["k"], inputs["v"], inputs["mask"]
    bm = _block_mask(mask)
    nc = _get_program(bm)
    waves, ns, _ = _build_schedule(bm)
    qTr, kTe, v2e, vp, kp = _prep_inputs(q, k, v, waves, ns)
    in_maps = []
    for c in range(NCORES):
        sl = slice(c * HPC, (c + 1) * HPC)
        in_maps.append({
            "qT": np.ascontiguousarray(qTr[sl]),
            "kT": np.ascontiguousarray(kTe[sl]),
            "v2": np.ascontiguousarray(v2e[sl]),
            "vp": np.ascontiguousarray(vp[sl]),
            "kp": np.ascontiguousarray(kp[sl]),
        })
    bkr = run_bass_kernel_spmd(nc, in_maps, list(range(NCORES)), trace=trace)
    pieces = []
    for r in bkr.results:
        for hh in range(HPC):
            accT = np.concatenate(
                [np.asarray(r[f"o_{hh}_{bb}"]) for bb in range(NBANK)],
                axis=1)                       # [65, 64*64] by position
            accT = accT.reshape(DA, NB, BLK)
            out_pos = (accT[:D] / accT[D:]).transpose(1, 2, 0)  # [NB,BLK,D]
            inv = np.asarray([QPOS[r2] for r2 in range(NB)])
            pieces.append(out_pos[inv].reshape(S, D))
    out = np.stack(pieces, axis=0).reshape(B, H, S, D).astype(np.float32)
    return out, bkr


def kernel(**inputs):
    out, _ = _run(inputs, trace=False)
    return out


# revision 26
# speedup vs baseline: 1.2771x; 1.0368x over previous
"""BigBird simulated attention on 8 Trainium2 NeuronCores.

Strategy
--------
B*H = 24 (batch, head) pairs are sharded 3-per-core across 8 cores (data/head
parallel). The BigBird mask is block-constant on 64x64 tiles, so the host
compresses it to a 64x64 block map and bakes a block-sparse schedule directly
into the instruction stream (the mask never goes to the device).

Per (head, q-block of 64 rows) scores are computed TRANSPOSED (S^T: k on
partitions, q on free):

  S^T[k, q] = sum_d K[k, d] Q[q, d]    (lhsT = K^T block cols, rhs = Q^T)
  P^T = exp(S^T / 8)                    (one ScalarE activation per wave)

PV runs with the V-pair as the STATIONARY operand and P^T as the moving one,
so one matmul serves every q-block of a unit at once:

  acc^T[:, q] += Vaug_pair^T @ P^T_pair[:, q]   with Vaug = [V | 1]

acc^T is [65, q]: row 64 is the softmax denominator (ones column of Vaug).
The division happens on the HOST after the un-normalized [65, q] tiles are
DMA'd back -- softmax is shift-invariant and exp can't overflow (scores
~N(0,1) after the 1/8 scale), so no max-subtraction is needed.

All matmuls are bf16 (tolerance 2e-2; this lands ~6e-3). The PE on this
toolchain serializes LDWEIGHTS with MATMUL (single weight buffer, ~90ns per
pair at these sizes), so the schedule minimizes MATMUL COUNT:
  - q-blocks are processed in device order PERM = [1..62, 0, 63],
  - the global (0,63) k-pair (every middle row attends it; appended
    host-side as resident pair #32) is ONE QK matmul + ONE PV matmul per
    7-row output bank,
  - window pairs (2t,2t+1) shared by adjacent rows 2t,2t+1 are one 128-wide
    QK + one PV,
  - full rows 0/63 (adjacent in device order, sharing all 32 aligned pairs)
    are one 128-wide QK + one PV per pair,
  - arbitrary leftover pairs are gathered host-side into per-chunk K-pair
    and V-pair tensors (kp/vp, streamed double-buffered per wave), so each
    is ONE QK matmul (128-partition) + ONE PV matmul.

Output PSUM banks hold acc^T [65, <=7 rows * 64]; banks are opened with a
[128,1] start=True dummy matmul that marks the bank's whole 2KB zero-region
pending on ALL partitions, after which every real PV matmul runs
start=False (first touch overwrites, later touches accumulate). Bank
position ranges: 8 banks of 7 middle rows, then [56..61], then [62,63]
(rows 0 and 63 share the last bank so their PV merges).

Sync: the Tile framework tracks all deps; after emission the Bacc passes
move_matmul_waits_to_ldweights + generate_event_semaphores re-establish the
TRN2 "at most one sync wait per instruction" constraint.
"""

import numpy as np

import concourse.bass as bass
import concourse.tile as tile
from concourse import mybir
from concourse.bass_utils import run_bass_kernel_spmd

B, H, S, D = 2, 12, 4096, 64
BLK = 64
NB = S // BLK            # 64 blocks per axis
DA = D + 1               # v plus ones column
NCORES = 8
HPC = B * H // NCORES    # heads per core
SCALE = 1.0 / 8.0        # 1/sqrt(64)
WAVE_CHUNKS = 24         # 24*64 cols = exactly 3 PSUM banks per score tile
NCHUNK = S // 128        # natural 128-row chunks of V
PAIR_G = NCHUNK          # resident pair index for the global (0, 63) pair
NPAIR = NCHUNK + 1

# q-block order on device: middle rows first, then the two full rows
PERM = list(range(1, NB - 1)) + [0, NB - 1]
QPOS = {r: p for p, r in enumerate(PERM)}
# output-bank position ranges: 8x7 middle rows, [56..61], then [62,63]
BANKS = [range(7 * b, 7 * b + 7) for b in range(8)] + [range(56, 62),
                                                       range(62, 64)]
NBANK = len(BANKS)
POSBANK = {p: b for b, rng in enumerate(BANKS) for p in rng}

F32 = mybir.dt.float32
BF16 = mybir.dt.bfloat16


# ----------------------------------------------------------------- schedule

def _block_mask(mask: np.ndarray) -> np.ndarray:
    m = np.asarray(mask).reshape(NB, BLK, NB, BLK)
    bm = m[:, 0, :, 0]
    assert bool(np.all(m == bm[:, None, :, None])), (
        "mask is not 64x64 block-constant; this kernel's schedule requires it"
    )
    return bm > 0


def _row_chunks(bm: np.ndarray, i: int):
    L = set(np.nonzero(bm[i])[0].tolist())
    full = len(L) == NB
    has_g = False
    if not full and 0 in L and NB - 1 in L:
        L -= {0, NB - 1}
        has_g = True
    aligned = [t for t in range(NB // 2) if 2 * t in L and 2 * t + 1 in L]
    cov = {b for t in aligned for b in (2 * t, 2 * t + 1)}
    singles = sorted(L - cov)
    spairs = [(singles[k], singles[k + 1] if k + 1 < len(singles) else None)
              for k in range(0, len(singles), 2)]
    return full, has_g, aligned, spairs


def _usize(u):
    if u[0] == "G":
        return u[2]
    return 2 if u[0] in ("W", "FR") else 1


def _ubank(u):
    if u[0] == "G":
        return POSBANK[u[1]]
    if u[0] == "W":
        return POSBANK[u[2]]
    if u[0] == "FR":
        return POSBANK[NB - 2]
    return POSBANK[u[1]]


def _build_units(bm: np.ndarray):
    info = {i: _row_chunks(bm, i) for i in range(NB)}
    wset = {}
    for t in range(NB // 2):
        r0, r1 = 2 * t, 2 * t + 1
        if (not info[r0][0] and not info[r1][0]
                and t in info[r0][2] and t in info[r1][2]):
            wset[t] = (r0, r1)
    units = []
    for b, prange in enumerate(BANKS):
        gpos = [p for p in prange if p < NB - 2]
        if gpos:
            assert all(info[PERM[p]][1] for p in gpos)
            units.append(("G", gpos[0], len(gpos)))
        for p in prange:
            r = PERM[p]
            full, has_g, aligned, spairs = info[r]
            if full:
                continue  # covered by FR units
            for t in aligned:
                if t in wset and r in wset[t]:
                    if r == wset[t][0]:
                        units.append(("W", t, QPOS[wset[t][0]]))
                else:
                    units.append(("P", p, t))
            for (gA, gB) in spairs:
                units.append(("S", p, gA, gB))
        if b == NBANK - 2:
            for t in range(NB // 2):
                units.append(("FR", t))
    return units


def _pack(units):
    """Pack units into 24-slot waves; multi-slot units must not cross an
    8-slot PSUM score-bank boundary. Lookahead picks are restricted to the
    head unit's bank or the next, so at most two output banks are ever
    accumulating at once (obpool bufs=2)."""
    pending = list(units)
    flat = []
    pos = 0
    while pending:
        rem = 8 - (pos % 8)
        head_bank = _ubank(pending[0])
        pick = None
        for idx in range(min(len(pending), 16)):
            u = pending[idx]
            if _usize(u) <= rem and _ubank(u) <= head_bank + 1:
                pick = idx
                break
        if pick is None:
            flat.append((pos, ("X",)))
            pos += 1
        else:
            u = pending.pop(pick)
            flat.append((pos, u))
            pos += _usize(u)
    waves = []
    for (p, u) in flat:
        w = p // WAVE_CHUNKS
        while len(waves) <= w:
            waves.append([])
        waves[w].append((p % WAVE_CHUNKS, u))
    return waves


def _unit_pv(u, slot):
    """PV matmuls for a unit: (pos0, pT slot0, width, source).

    source: ("v2", pair) resident, or ("vp", sidx) gathered."""
    k = u[0]
    if k == "G":
        return [(u[1], slot, u[2], ("v2", PAIR_G))]
    if k == "FR":
        return [(NB - 2, slot, 2, ("v2", u[1]))]
    if k == "W":
        p0 = u[2]
        if POSBANK[p0] == POSBANK[p0 + 1]:
            return [(p0, slot, 2, ("v2", u[1]))]
        return [(p0, slot, 1, ("v2", u[1])),
                (p0 + 1, slot + 1, 1, ("v2", u[1]))]
    if k == "P":
        return [(u[1], slot, 1, ("v2", u[2]))]
    return [(u[1], slot, 1, ("vp", u[4]))]


def _build_schedule(bm: np.ndarray):
    units = _build_units(bm)
    waves = _pack(units)
    ns = 0
    waves2 = []
    pos_chunks = np.zeros(NB, dtype=np.int64)
    for wave in waves:
        w2 = []
        for slot, u in wave:
            if u[0] == "S":
                u = u + (ns,)
                ns += 1
            w2.append((slot, u))
            if u[0] != "X":
                for (p0, s0, width, src) in _unit_pv(u, slot):
                    for j in range(width):
                        pos_chunks[p0 + j] += 1
        waves2.append(w2)
    return waves2, ns, pos_chunks


# ------------------------------------------------------------------ program

def _emit_head(tc, pools, h, waves, pos_chunks, qT_d, kT_d, v2_d, vp_d, kp_d,
               o_d):
    nc = tc.nc
    (wq, wk, wv, vppool, kppool, ppool, stpool, obpool, fpool) = pools

    qT = wq.tile([64, S], BF16, tag="qT", name=f"qT{h}")
    kT = wk.tile([64, S + 2 * BLK], BF16, tag="kT", name=f"kT{h}")
    v2 = wv.tile([128, NPAIR * DA], BF16, tag="v2", name=f"v2_{h}")
    vps = [vppool.tile([128, WAVE_CHUNKS * DA], BF16, tag=f"vp{j}",
                       name=f"vp{j}h{h}") for j in range(3)]
    kps = [kppool.tile([64, WAVE_CHUNKS * 128], BF16, tag=f"kp{j}",
                       name=f"kp{j}h{h}") for j in range(3)]
    # big per-head loads go on the SWDGE queue (few, issue rate moot);
    # the frequent per-wave kp/vp loads ride the SP HWDGE queue
    nc.gpsimd.dma_start(out=qT, in_=qT_d[h])
    nc.gpsimd.dma_start(out=kT, in_=kT_d[h])
    nc.gpsimd.dma_start(out=v2, in_=v2_d[h])

    ob_tiles = {}
    slots_left = {b: int(sum(pos_chunks[p] for p in rng))
                  for b, rng in enumerate(BANKS)}

    def _finalize(bank):
        ob = ob_tiles.pop(bank)
        nq = len(BANKS[bank])
        oc = fpool.tile([128, 7 * BLK], F32, tag="oc", name=f"och{h}_{bank}")
        nc.vector.tensor_copy(oc[0:DA, : nq * BLK], ob[0:DA, : nq * BLK])
        nc.scalar.dma_start(out=o_d[h][bank][:, :],
                            in_=oc[0:DA, : nq * BLK])

    def emit_qk(wave, st, kp, k0):
        for slot, u in wave:
            c0 = slot * BLK
            kind = u[0]
            if kind == "X":
                continue
            if kind == "G":
                p0, g = u[1], u[2]
                nc.tensor.matmul(
                    st[:, c0: c0 + g * BLK],
                    lhsT=kT[0:64, 2 * PAIR_G * BLK: (2 * PAIR_G + 2) * BLK],
                    rhs=qT[0:64, p0 * BLK: (p0 + g) * BLK],
                    start=True, stop=True, skip_group_check=True)
            elif kind in ("W", "FR"):
                t = u[1]
                p0 = u[2] if kind == "W" else NB - 2
                nc.tensor.matmul(
                    st[:, c0: c0 + 2 * BLK],
                    lhsT=kT[0:64, 2 * t * BLK: (2 * t + 2) * BLK],
                    rhs=qT[0:64, p0 * BLK: (p0 + 2) * BLK],
                    start=True, stop=True, skip_group_check=True)
            elif kind == "P":
                p, t = u[1], u[2]
                nc.tensor.matmul(
                    st[:, c0: c0 + BLK],
                    lhsT=kT[0:64, 2 * t * BLK: (2 * t + 2) * BLK],
                    rhs=qT[0:64, p * BLK: (p + 1) * BLK],
                    start=True, stop=True, skip_group_check=True)
            else:  # S: gathered K pair, one 128-partition matmul
                p, li = u[1], u[4] - k0
                nc.tensor.matmul(
                    st[:, c0: c0 + BLK],
                    lhsT=kp[0:64, li * 128: (li + 1) * 128],
                    rhs=qT[0:64, p * BLK: (p + 1) * BLK],
                    start=True, stop=True, skip_group_check=True)

    def emit_pv(wave, pT, vp, s0):
        for slot, u in wave:
            if u[0] == "X":
                continue
            for (p0, ps, width, src) in _unit_pv(u, slot):
                bank = POSBANK[p0]
                if bank not in ob_tiles:
                    ob = obpool.tile([128, 512], F32, tag="ob",
                                     name=f"obh{h}_{bank}")
                    ob_tiles[bank] = ob
                    # start=True [128,1]: marks the whole 2KB zero-region
                    # pending on ALL partitions; real PVs run start=False
                    nc.tensor.matmul(
                        ob[0:128, 508:509], lhsT=qT[0:1, 0:128],
                        rhs=qT[0:1, 0:1],
                        start=True, stop=True, skip_group_check=True)
                ob = ob_tiles[bank]
                if src[0] == "v2":
                    lhsT = v2[:, src[1] * DA: (src[1] + 1) * DA]
                else:
                    li = src[1] - s0
                    lhsT = vp[:, li * DA: (li + 1) * DA]
                col0 = (p0 - BANKS[bank].start) * BLK
                slots_left[bank] -= width
                nc.tensor.matmul(
                    ob[0:DA, col0: col0 + width * BLK],
                    lhsT=lhsT,
                    rhs=pT[:, ps * BLK: (ps + width) * BLK],
                    start=False, stop=(slots_left[bank] == 0),
                    skip_group_check=True)
                if slots_left[bank] == 0:
                    # eager: free the psum bank before the next one opens
                    del slots_left[bank]
                    _finalize(bank)

    def wave_s0(j):
        sidxs = [u[4] for _, u in waves[j] if u[0] == "S"]
        return min(sidxs) if sidxs else 0

    def load_sp(j):
        sidxs = [u[4] for _, u in waves[j] if u[0] == "S"]
        if not sidxs:
            return
        s0, scnt = min(sidxs), len(sidxs)
        assert sidxs == list(range(s0, s0 + scnt))
        # contiguous column slices: one ~1-2KB run per partition
        nc.sync.dma_start(
            out=vps[j % 3][:, : scnt * DA],
            in_=vp_d[h][:, s0 * DA: (s0 + scnt) * DA])
        nc.sync.dma_start(
            out=kps[j % 3][:, : scnt * 128],
            in_=kp_d[h][:, s0 * 128: (s0 + scnt) * 128])

    for j in range(min(3, len(waves))):
        load_sp(j)
    prev = None
    for w, wave in enumerate(waves):
        st = stpool.tile([128, WAVE_CHUNKS * BLK], F32, tag="st",
                         name=f"sth{h}_{w}")
        emit_qk(wave, st, kps[w % 3], wave_s0(w))
        pT = ppool.tile([128, WAVE_CHUNKS * BLK], BF16, tag="pT",
                        name=f"pTh{h}_{w}")
        ncols = (wave[-1][0] + _usize(wave[-1][1])) * BLK
        nc.scalar.activation(
            out=pT[:, :ncols], in_=st[:, :ncols],
            func=mybir.ActivationFunctionType.Exp, scale=SCALE)
        if prev is not None:
            emit_pv(waves[prev[0]], prev[1], prev[2], wave_s0(prev[0]))
            if w + 2 < len(waves):
                load_sp(w + 2)
        prev = (w, pT, vps[w % 3])
    emit_pv(waves[prev[0]], prev[1], prev[2], wave_s0(prev[0]))
    for bank in sorted(ob_tiles):
        slots_left.pop(bank, None)
        _finalize(bank)


def _build_program(bm: np.ndarray):
    import os as _os
    hpc = int(_os.environ.get("BB_HPC", HPC))
    waves, ns, pos_chunks = _build_schedule(bm)
    nc = bass.Bass("TRN2", target_bir_lowering=False, debug=False,
                   enable_asserts=False)
    qT_d = nc.dram_tensor("qT", [HPC, 64, S], BF16, kind="ExternalInput")
    kT_d = nc.dram_tensor("kT", [HPC, 64, S + 2 * BLK], BF16,
                          kind="ExternalInput")
    v2_d = nc.dram_tensor("v2", [HPC, 128, NPAIR * DA], BF16,
                          kind="ExternalInput")
    vp_d = nc.dram_tensor("vp", [HPC, 128, max(ns, 1) * DA], BF16,
                          kind="ExternalInput")
    kp_d = nc.dram_tensor("kp", [HPC, 64, max(ns, 1) * 128], BF16,
                          kind="ExternalInput")
    o_d = [[nc.dram_tensor(f"o_{hh}_{bb}", [DA, len(BANKS[bb]) * BLK], F32,
                           kind="ExternalOutput")
            for bb in range(NBANK)] for hh in range(HPC)]

    with tile.TileContext(nc) as tc:
        with (
            tc.tile_pool(name="wq", bufs=HPC) as wq,
            tc.tile_pool(name="wk", bufs=HPC) as wk,
            tc.tile_pool(name="wv", bufs=HPC) as wv,
            tc.tile_pool(name="vpp", bufs=HPC) as vppool,
            tc.tile_pool(name="kpp", bufs=HPC) as kppool,
            tc.tile_pool(name="pT", bufs=3) as ppool,
            tc.tile_pool(name="st", bufs=2, space="PSUM") as stpool,
            tc.tile_pool(name="ob", bufs=2, space="PSUM") as obpool,
            tc.tile_pool(name="fin", bufs=3) as fpool,
        ):
            pools = (wq, wk, wv, vppool, kppool, ppool, stpool, obpool,
                     fpool)
            for hh in range(hpc):
                _emit_head(tc, pools, hh, waves, pos_chunks,
                           qT_d, kT_d, v2_d, vp_d, kp_d, o_d)
    import bass_rust as _bass_rust
    _bass_rust.move_matmul_waits_to_ldweights(nc.m)
    _bass_rust.generate_event_semaphores(nc)
    return nc


_CACHE = {}


def _get_program(bm: np.ndarray):
    key = bm.tobytes()
    if key not in _CACHE:
        _CACHE[key] = _build_program(bm)
    return _CACHE[key]


# -------------------------------------------------------------------- entry

def _prep_inputs(q, k, v, waves, ns):
    import ml_dtypes
    bf16 = ml_dtypes.bfloat16
    q = np.ascontiguousarray(np.asarray(q), dtype=np.float32)
    k = np.ascontiguousarray(np.asarray(k), dtype=np.float32)
    v = np.ascontiguousarray(np.asarray(v), dtype=np.float32)
    qT = q.reshape(B * H, S, D).transpose(0, 2, 1).astype(bf16)
    qTr = np.ascontiguousarray(
        qT.reshape(B * H, D, NB, BLK)[:, :, PERM, :].reshape(B * H, D, S))
    kT = k.reshape(B * H, S, D).transpose(0, 2, 1).astype(bf16)
    kTe = np.ascontiguousarray(np.concatenate(
        [kT, kT[:, :, :BLK], kT[:, :, (NB - 1) * BLK:]], axis=2))
    vA = np.concatenate(
        [v.reshape(B * H, S, D),
         np.ones((B * H, S, 1), dtype=np.float32)], axis=2).astype(bf16)
    v2 = vA.reshape(B * H, NCHUNK, 128, DA)
    vblk = vA.reshape(B * H, NB, BLK, DA)
    pair_g = np.concatenate([vblk[:, 0], vblk[:, NB - 1]], axis=1)
    v2e = np.ascontiguousarray(
        np.concatenate([v2, pair_g[:, None]], axis=1)
        .transpose(0, 2, 1, 3).reshape(B * H, 128, NPAIR * DA))
    kblk = kTe[:, :, :S].reshape(B * H, D, NB, BLK)
    vp = np.zeros((B * H, max(ns, 1), 128, DA), dtype=bf16)
    kp = np.zeros((B * H, max(ns, 1), 64, 128), dtype=bf16)
    for wave in waves:
        for slot, u in wave:
            if u[0] != "S":
                continue
            for half, g in enumerate(u[2:4]):
                if g is not None:
                    vp[:, u[4], half * 64: half * 64 + 64, :] = vblk[:, g]
                    kp[:, u[4], :, half * 64: half * 64 + 64] = \
                        kblk[:, :, g, :]
    # partition-major layouts so per-wave loads are contiguous column runs
    vp = np.ascontiguousarray(
        vp.transpose(0, 2, 1, 3).reshape(B * H, 128, -1))
    kp = np.ascontiguousarray(
        kp.transpose(0, 2, 1, 3).reshape(B * H, 64, -1))
    return qTr, kTe, v2e, vp, kp


def _run(inputs, trace=False):
    q, k, v, mask = inputs["q"], inputs["k"], inputs["v"], inputs["mask"]
    bm = _block_mask(mask)
    nc = _get_program(bm)
    waves, ns, _ = _build_schedule(bm)
    qTr, kTe, v2e, vp, kp = _prep_inputs(q, k, v, waves, ns)
    in_maps = []
    for c in range(NCORES):
        sl = slice(c * HPC, (c + 1) * HPC)
        in_maps.append({
            "qT": np.ascontiguousarray(qTr[sl]),
            "kT": np.ascontiguousarray(kTe[sl]),
            "v2": np.ascontiguousarray(v2e[sl]),
            "vp": np.ascontiguousarray(vp[sl]),
            "kp": np.ascontiguousarray(kp[sl]),
        })
    bkr = run_bass_kernel_spmd(nc, in_maps, list(range(NCORES)), trace=trace)
    pieces = []
    for r in bkr.results:
        for hh in range(HPC):
            accT = np.concatenate(
                [np.asarray(r[f"o_{hh}_{bb}"]) for bb in range(NBANK)],
                axis=1)                       # [65, 64*64] by position
            accT = accT.reshape(DA, NB, BLK)
            out_pos = (accT[:D] / accT[D:]).transpose(1, 2, 0)  # [NB,BLK,D]
            inv = np.asarray([QPOS[r2] for r2 in range(NB)])
            pieces.append(out_pos[inv].reshape(S, D))
    out = np.stack(pieces, axis=0).reshape(B, H, S, D).astype(np.float32)
    return out, bkr


def kernel(**inputs):
    out, _ = _run(inputs, trace=False)
    return out
